# revision 12
# baseline (speedup 1.0000x reference)
import sys
import functools

sys.path.insert(0, "/opt/trn_rl_repo")
import numpy as np
import ml_dtypes

# nn_Causal_GraphConvolution, band-structured GAT kernel.
#
# Math: att = softmax_row(mask(adj, relu(wh1[r]+wh2[m]))); exp(relu(e)) =
# max(u[r]*v[m], 1) with u=exp(wh1), v=exp(wh2). Per (row r, node m):
#   p[m,r] = adj * max(u*v, 1)
# Host sorts, per (core, k): attention-row set = the core's pv0 block,
# rows ordered within the core by u_k; columns/chunks ordered by v_k.
# Then for a (chunk, 128-row slice): u*v < 1 everywhere below a band and
# > 1 above it, so p-slices collapse to:
#   FLOOR: p = adj                -> matmul(adj, whp)        into ps_flr
#   LIN:   p = adj*u*v            -> matmul(adj, v*whp)      into ps_lin,
#                                    final h' += u[r]*ps_lin
#   BAND (straddled): exact p = adj*max(uv,1) via DVE ts+tt (max form,
#         1 matmul) or ACT relu(uv-1) + tt (q=adj*t) + extra adj matmul
#         (relu form, 2 matmuls); greedy-balanced across DVE/ACT.
# Adjacency is fp8e4 (0/1 exact): copy-0 (k=0 orders, resident; reused by
# phase 2 for both k), copy-1 (k=1 orders, streamed). h' rows are indirect-
# scattered back to pv0 order before the AllGather so phase 2 is affine.
K = 2
N = 8192
IN_F = 128
OUT_F = 64
NCORES = 8
ROWS = N // NCORES
NCH = N // 128
RCH = ROWS // 128
CPG = 4
NGRP = NCH // CPG
DELTA = 0.015       # host-vs-device wh margin for band classification
ACT_TS_NS = 293.0   # ScalarE relu tile [128,128]
DVE_TS_NS = 94.0    # VectorE max-form ts tile [128,128]
DVE_TT_NS = 194.0   # VectorE mask tt tile [128,128] (fp8 in1, 1x)
DVE_MISC_NS = 42000.0
ACT_MISC_NS = 27000.0

_f8 = ml_dtypes.float8_e4m3
_bf = ml_dtypes.bfloat16


def _plan(x, weight, a):
    """Host-side sort/band planning. Returns the per-core in_map pieces and
    the (SPMD-uniform) band schedule."""
    w32 = np.asarray(weight, np.float32)
    a32 = np.asarray(a, np.float32)
    x32 = np.asarray(x, np.float32)
    wa1 = w32 @ a32[:OUT_F, 0]     # [IN_F]
    wa2 = w32 @ a32[OUT_F:, 0]
    wh1 = x32 @ wa1                # [K, N]
    wh2 = x32 @ wa2                # [K, N]

    # per k: global column (v) sort. Row sets: core c owns the c-th pv0
    # block (so its h' rows scatter back into its own AllGather window),
    # ordered within the core by u_k for the band trick.
    pv = [np.argsort(wh2[k], kind="stable") for k in range(K)]
    rowsets = []
    for k in range(K):
        rk = []
        for c in range(NCORES):
            base = pv[0][c * ROWS:(c + 1) * ROWS]
            rk.append(base[np.argsort(wh1[k][base], kind="stable")])
        rowsets.append(rk)

    # chunk v-ranges and band rows per (k, ch); r bounds uniform over cores
    bands = []
    for k in range(K):
        wh2s = wh2[k][pv[k]]
        ch_lo = wh2s.reshape(NCH, 128).min(axis=1)
        ch_hi = wh2s.reshape(NCH, 128).max(axis=1)
        bk = []
        for ch in range(NCH):
            r_lo = ROWS
            r_hi = 0
            for c in range(NCORES):
                u_s = wh1[k][rowsets[k][c]]
                r_lo = min(r_lo, int(np.searchsorted(u_s, -ch_hi[ch] - DELTA)))
                r_hi = max(r_hi, int(np.searchsorted(u_s, -ch_lo[ch] + DELTA)))
            bk.append((r_lo, r_hi))
        bands.append(bk)
    return pv, rowsets, bands, wh1, wh2


def _schedule(bands):
    """Slice classification + greedy DVE/ACT band-form assignment.
    Returns cls[k][ch][ns] in {'F','L','D','A'} (floor/lin/dve-band/act-band)."""
    act_t, dve_t = ACT_MISC_NS, DVE_MISC_NS
    cls = [[[None] * RCH for _ in range(NCH)] for _ in range(K)]
    for ch in range(NCH):
        for k in range(K):
            r_lo, r_hi = bands[k][ch]
            s_lo = r_lo // 128
            s_hi = min((r_hi + 127) // 128, RCH)
            w = max(s_hi - s_lo, 0)  # band span in slices
            # one ts+tt over the whole span; choose engine for the ts
            act_c = ACT_TS_NS * w
            dve_c = DVE_TS_NS * w
            tt_c = DVE_TT_NS * w
            if w and act_t + act_c < dve_t + dve_c:
                form = "A"
                act_t += act_c
                dve_t += tt_c
            else:
                form = "D"
                dve_t += dve_c + tt_c
            for ns in range(RCH):
                if ns < s_lo:
                    cls[k][ch][ns] = "F"
                elif ns >= s_hi:
                    cls[k][ch][ns] = "L"
                else:
                    cls[k][ch][ns] = form
    return cls


@functools.lru_cache(maxsize=2)
def _build(sched_key, timing=False):
    import concourse.bacc as bacc
    import concourse.tile as tile
    import concourse.bass as bass
    from concourse.tile import add_dep_helper
    from concourse import mybir

    cls = _build_sched  # set by _run before calling _build

    bf16 = mybir.dt.bfloat16
    f32 = mybir.dt.float32
    f8 = mybir.dt.float8e4
    i32 = mybir.dt.int32
    AO = mybir.AluOpType
    AF = mybir.ActivationFunctionType

    nc = bacc.Bacc(num_devices=NCORES)

    # adjP[k]: [NGRP, 128, CPG, ROWS] fp8; chunk ch=g*CPG+j at [g, :, j, :]
    adjP = [
        nc.declare_dram_parameter(f"adjP{k}", [NGRP, 128, CPG, ROWS], f8, False)
        for k in range(K)
    ]
    xT = nc.declare_dram_parameter("xT", [K, IN_F, N], bf16, False)
    xrT = nc.declare_dram_parameter("xrT", [K, IN_F, ROWS], bf16, False)
    waug = nc.declare_dram_parameter("waug", [IN_F, 66], bf16, False)
    # scatter indices: hp_acc row (ns*128+p) -> local position in hp_local[k]
    idxS = nc.declare_dram_parameter("idxS", [128, K, RCH], i32, False)
    out = nc.declare_dram_parameter("out", [K * OUT_F, ROWS], f32, True)

    urow = nc.dram_tensor("urow", [K, 1, ROWS], bf16)
    hp_local = [
        nc.dram_tensor(f"hp_local{k}", [ROWS, OUT_F], f8) for k in range(K)
    ]
    hp_full = [
        nc.dram_tensor(
            f"hp_full{k}", [N, OUT_F], f8,
            addr_space="Local" if timing else "Shared",
        )
        for k in range(K)
    ]

    with tile.TileContext(nc) as tc:
        with (
            tc.tile_pool(name="persist", bufs=1) as persist,
            tc.tile_pool(name="adj0p", bufs=NGRP) as adj0p,
            tc.tile_pool(name="adj1p", bufs=6) as adj1p,
            tc.tile_pool(name="xp", bufs=1) as xp,
            tc.tile_pool(name="tp", bufs=6) as tp,
            tc.tile_pool(name="pp", bufs=7) as pp,
            tc.tile_pool(name="hpio", bufs=4) as hpio,
            tc.tile_pool(name="sm", bufs=8) as sm,
        ):
            waug_sb = persist.tile([IN_F, 66], bf16, tag="waug")
            nc.scalar.dma_start(out=waug_sb, in_=waug[:])
            neg1 = persist.tile([128, 1], f32, tag="neg1")
            nc.vector.memset(neg1, -1.0)
            idx_sb = persist.tile([128, K, RCH], i32, tag="idx")
            nc.scalar.dma_start(out=idx_sb, in_=idxS[:])
            zero_w = persist.tile([128, 128], bf16, tag="zerow")
            nc.vector.memset(zero_w, 0.0)

            XQ = N // 4
            xbigs, xrs = [], []
            x_dmas = []
            for k in range(K):
                xb_k = []
                for q in range(4):
                    xbig = xp.tile([128, XQ], bf16, tag="xbig", bufs=2,
                                   name=f"xbig{k}_{q}")
                    x_dmas.append(nc.sync.dma_start(
                        out=xbig, in_=xT[k, :, q * XQ:(q + 1) * XQ]
                    ))
                    xb_k.append(xbig)
                xbigs.append(xb_k)
                xr_sb = xp.tile([128, ROWS], bf16, tag="xr", bufs=2,
                                name=f"xr{k}")
                nc.scalar.dma_start(out=xr_sb, in_=xrT[k])
                xrs.append(xr_sb)

            # ---- adjacency copy-0: resident (phase-1 k0 + phase-2 both k)
            adj0_sb = []
            adj0_dmas = []
            for g in range(NGRP):
                at = adj0p.tile([128, CPG, ROWS], f8, tag="adj0",
                                name=f"adj0_{g}")
                d = nc.sync.dma_start(out=at, in_=adjP[0][g])
                if g >= 2:
                    add_dep_helper(d.ins, adj0_dmas[g - 2].ins,
                                   reason="stream copy-0 in order")
                else:
                    add_dep_helper(d.ins, x_dmas[-1].ins,
                                   reason="x lands before adjacency")
                adj0_dmas.append(d)
                adj0_sb.append(at)

            def adj0_ch(ch):
                return adj0_sb[ch // CPG][:, ch % CPG, :]

            # ---- phase 0 per k: whp, whpv, v, u ----
            whp, whpv, v_sb, u_bc, u_col = [], [], [], [], []
            psA_cm = tc.tile_pool(name="psA", bufs=3, space="PSUM")
            psA = psA_cm.__enter__()
            for k in range(K):
                xr_sb = xrs[k]
                for half in range(2):
                    psu = psA.tile([1, 512], f32, tag="psu",
                                   name=f"psu{k}_{half}")
                    nc.tensor.matmul(
                        psu,
                        lhsT=waug_sb[:, 64:65],
                        rhs=xr_sb[:, half * 512:(half + 1) * 512],
                        start=True, stop=True,
                    )
                    uh = sm.tile([1, 512], bf16, tag="uh", name=f"uh{k}_{half}")
                    nc.scalar.activation(uh, psu, AF.Exp)
                    nc.gpsimd.dma_start(
                        out=urow[k, :, half * 512:(half + 1) * 512], in_=uh
                    )
                ub = persist.tile([128, ROWS], bf16, tag=f"ub{k}")
                nc.gpsimd.dma_start(out=ub, in_=urow[k].to_broadcast((128, ROWS)))
                u_bc.append(ub)
                ucT = persist.tile([128, RCH], bf16, tag=f"uc{k}")
                nc.gpsimd.dma_start(
                    out=ucT, in_=urow[k].rearrange("o (ns p) -> (o p) ns", p=128)
                )
                ucTf = persist.tile([128, RCH], f32, tag=f"ucf{k}")
                nc.vector.tensor_copy(ucTf, ucT)
                u_col.append(ucTf)

                whp_k = persist.tile([128, NCH, 65], bf16, tag=f"whp{k}")
                nc.vector.memset(whp_k[:, :, 64:65], 1.0)
                whpv_k = persist.tile([128, NCH, 65], bf16, tag=f"whpv{k}")
                wh2_k = persist.tile([128, NCH], f32, tag=f"wh2{k}")
                for q in range(4):
                    xbig = xbigs[k][q]
                    for cb in range(0, NCH // 4, 4):
                        ps0 = psA.tile([128, 4, 66], f32, tag="ps0",
                                       name=f"ps0_{k}_{q}_{cb}")
                        for j in range(4):
                            ch = cb + j
                            nc.tensor.matmul(
                                ps0[:, j, :],
                                lhsT=xbig[:, ch * 128:(ch + 1) * 128],
                                rhs=waug_sb,
                                start=True, stop=True,
                            )
                        gch = q * (NCH // 4) + cb
                        nc.scalar.copy(
                            whp_k[:, gch:gch + 4, 0:64], ps0[:, :, 0:64]
                        )
                        nc.vector.tensor_copy(wh2_k[:, gch:gch + 4], ps0[:, :, 65])
                whp.append(whp_k)

                v_k = persist.tile([128, NCH], f32, tag=f"v{k}")
                nc.scalar.activation(
                    v_k[:, 0:NCH // 2], wh2_k[:, 0:NCH // 2], AF.Exp
                )
                nc.scalar.activation(
                    v_k[:, NCH // 2:], wh2_k[:, NCH // 2:], AF.Exp
                )
                v_sb.append(v_k)
                # whpv = v * whp (incl. ones col -> v), per chunk
                for ch in range(NCH):
                    nc.vector.tensor_scalar_mul(
                        whpv_k[:, ch, :], whp_k[:, ch, :], v_k[:, ch:ch + 1]
                    )
                whpv.append(whpv_k)
            psA_cm.__exit__(None, None, None)

            # ---- adjacency copy-1 stream starts after copy-0 ----
            adj1_dmas = {}

            def adj1_load(g):
                at = adj1p.tile([128, CPG, ROWS], f8, tag="adj1",
                                name=f"adj1_{g}")
                d = nc.sync.dma_start(out=at, in_=adjP[1][g])
                if g == 0:
                    add_dep_helper(d.ins, adj0_dmas[-1].ins,
                                   reason="copy-1 after copy-0")
                adj1_dmas[g] = d
                return at

            ag_insts = []
            psF_cm = tc.tile_pool(name="psF", bufs=2, space="PSUM")
            psF = psF_cm.__enter__()
            psL_cm = tc.tile_pool(name="psL", bufs=2, space="PSUM")
            psL = psL_cm.__enter__()

            # ---- phase 1 per k ----
            for k in range(K):
                if k == 1:
                    adj1_sb = {}

                def adj_ch_k(ch):
                    if k == 0:
                        return adj0_ch(ch)
                    g = ch // CPG
                    if g not in adj1_sb:
                        adj1_sb[g] = adj1_load(g)
                        adj1_sb.pop(g - 5, None)
                    return adj1_sb[g][:, ch % CPG, :]

                # bank-safe: [128, 4, 65] f32 = 1040B per tile (one bank),
                # two tiles per family. start=True clears has_written for the
                # WHOLE bank, so: one start=True zero-init MM per tile, then
                # zero-init the other 3 slices (start=False overwrites since
                # has_written was cleared), then all real MMs accumulate with
                # start=False. stop=True on the last MM into each tile.
                ps_flr_t = [
                    psF.tile([128, 4, 65], f32, tag="accf", name=f"psf{k}_{h}")
                    for h in range(2)
                ]
                ps_lin_t = [
                    psL.tile([128, 4, 65], f32, tag="accl", name=f"psl{k}_{h}")
                    for h in range(2)
                ]
                ps_flr = [ps_flr_t[i // 4][:, i % 4, :] for i in range(RCH)]
                ps_lin = [ps_lin_t[i // 4][:, i % 4, :] for i in range(RCH)]
                # last real MM per (family, tile-half): (ch, ns, tag)
                last_mm = {}
                for ch in range(NCH):
                    for ns in range(RCH):
                        c = cls[k][ch][ns]
                        fam = "L" if c == "L" else "F"
                        tag = "F2" if c == "A" else c[0] if c != "D" else "D"
                        last_mm[(fam, ns // 4)] = (ch, ns, c)
                for tiles in (ps_flr, ps_lin):
                    for i in range(RCH):
                        nc.tensor.matmul(
                            tiles[i], lhsT=zero_w, rhs=whp[k][:, 0, :],
                            start=(i % 4 == 0), stop=False,
                            skip_group_check=True,
                        )

                def fl_flags(fam, ch, ns, c):
                    sp = last_mm.get((fam, ns // 4)) == (ch, ns, c)
                    return dict(start=False, stop=sp, skip_group_check=True)

                for ch in range(NCH):
                    a_ch = adj_ch_k(ch)
                    # one band-spanning ts+tt for this chunk's straddled slices
                    sset = [ns for ns in range(RCH) if cls[k][ch][ns] in "DA"]
                    p_band = None
                    if sset:
                        b_lo, b_hi = sset[0], sset[-1] + 1
                        bw = (b_hi - b_lo) * 128
                        bsl = slice(b_lo * 128, b_hi * 128)
                        form = cls[k][ch][sset[0]]
                        t = tp.tile([128, bw], bf16, tag="t",
                                    name=f"t{k}_{ch}")
                        if form == "A":
                            nc.scalar.activation(
                                t, u_bc[k][:, bsl], AF.Relu,
                                bias=neg1, scale=v_sb[k][:, ch:ch + 1],
                            )
                        else:
                            nc.vector.tensor_scalar(
                                out=t, in0=u_bc[k][:, bsl],
                                scalar1=v_sb[k][:, ch:ch + 1],
                                scalar2=1.0, op0=AO.mult, op1=AO.max,
                            )
                        p_band = pp.tile([128, bw], bf16, tag="p",
                                         name=f"p{k}_{ch}")
                        nc.vector.tensor_mul(p_band, t, a_ch[:, bsl])
                    for ns in range(RCH):
                        c = cls[k][ch][ns]
                        sl = slice(ns * 128, (ns + 1) * 128)
                        psl = slice((ns - (sset[0] if sset else 0)) * 128,
                                    (ns - (sset[0] if sset else 0)) * 128 + 128)
                        if c == "F":
                            nc.tensor.matmul(
                                ps_flr[ns], lhsT=a_ch[:, sl],
                                rhs=whp[k][:, ch, :],
                                **fl_flags("F", ch, ns, c),
                            )
                        elif c == "L":
                            nc.tensor.matmul(
                                ps_lin[ns], lhsT=a_ch[:, sl],
                                rhs=whpv[k][:, ch, :],
                                **fl_flags("L", ch, ns, c),
                            )
                        elif c == "D":
                            nc.tensor.matmul(
                                ps_flr[ns], lhsT=p_band[:, psl],
                                rhs=whp[k][:, ch, :],
                                **fl_flags("F", ch, ns, c),
                            )
                        else:  # 'A': relu form, two matmuls
                            nc.tensor.matmul(
                                ps_flr[ns], lhsT=a_ch[:, sl],
                                rhs=whp[k][:, ch, :],
                                start=False, stop=False,
                                skip_group_check=True,
                            )
                            nc.tensor.matmul(
                                ps_flr[ns], lhsT=p_band[:, psl],
                                rhs=whp[k][:, ch, :],
                                **fl_flags("F", ch, ns, c),
                            )

                # combine + normalize
                hp_acc = hpio.tile([128, RCH, OUT_F], f8, tag="hpacc",
                                   bufs=2, name=f"hpacc{k}")
                for ns in range(RCH):
                    comb = sm.tile([128, 65], f32, tag="comb",
                                   name=f"comb{k}_{ns}")
                    nc.vector.tensor_scalar_mul(
                        comb, ps_lin[ns], u_col[k][:, ns:ns + 1]
                    )
                    nc.vector.tensor_add(comb, comb, ps_flr[ns])
                    rs = sm.tile([128, 1], f32, tag="rs", name=f"rs{k}_{ns}")
                    nc.vector.reciprocal(rs, comb[:, 64:65])
                    nc.vector.tensor_scalar_mul(
                        hp_acc[:, ns, :], comb[:, 0:64], rs
                    )
                for ns in range(RCH):
                    nc.gpsimd.indirect_dma_start(
                        out=hp_local[k][:],
                        out_offset=bass.IndirectOffsetOnAxis(
                            ap=idx_sb[:, k, ns:ns + 1], axis=0
                        ),
                        in_=hp_acc[:, ns, :],
                        in_offset=None,
                    )
                if timing:
                    nc.scalar.dma_start(
                        out=hp_full[k][0:ROWS, :], in_=hp_local[k][:]
                    )
                else:
                    ag = nc.gpsimd.collective_compute(
                        "AllGather",
                        mybir.AluOpType.bypass,
                        replica_groups=[list(range(NCORES))],
                        ins=[hp_local[k][:]],
                        outs=[hp_full[k][:]],
                    )
                    ag_insts.append(ag)

            # ---- phase 2: out^T = relu(hp^T @ adj0^T), copy-0, both k ----
            psL_cm.__exit__(None, None, None)
            psF_cm.__exit__(None, None, None)
            psO_cm = tc.tile_pool(name="psO", bufs=2, space="PSUM")
            psO = psO_cm.__enter__()
            ps_o = [
                psO.tile([128, 512], f32, tag="acco", name=f"pso{h}")
                for h in range(2)
            ]
            out_acc = hpio.tile([128, ROWS], f32, tag="outacc", bufs=1)
            for g in range(NGRP):
                hpg = hpio.tile([128, CPG, K, OUT_F], f8, tag="hpg",
                                name=f"hpg{g}")
                base = g * CPG * 128
                for k in range(K):
                    nc.sync.dma_start(
                        out=hpg[:, :, k, :],
                        in_=hp_full[k][base:base + CPG * 128, :].rearrange(
                            "(j p) o -> p j o", p=128
                        ),
                    )
                for j in range(CPG):
                    ch = g * CPG + j
                    for h in range(2):
                        nc.tensor.matmul(
                            ps_o[h],
                            lhsT=hpg[:, j, :, :],
                            rhs=adj0_ch(ch)[:, h * 512:(h + 1) * 512],
                            start=(ch == 0),
                            stop=(ch == NCH - 1),
                        )
            for h in range(2):
                nc.vector.tensor_scalar_max(
                    out_acc[:, h * 512:(h + 1) * 512], ps_o[h], 0.0
                )
            nc.sync.dma_start(out=out[:], in_=out_acc)
            psO_cm.__exit__(None, None, None)

    nc.finalize()
    return nc


_build_sched = None


def _prep(x, adj, weight, a):
    pv, rowsets, bands, wh1, wh2 = _plan(x, weight, a)
    cls = _schedule(bands)

    w32 = np.asarray(weight, np.float32)
    a32 = np.asarray(a, np.float32)
    waug = np.concatenate(
        [w32, w32 @ a32[:OUT_F], w32 @ a32[OUT_F:]], axis=1
    ).astype(_bf)
    x32 = np.asarray(x, np.float32)
    adj8 = np.asarray(adj, np.float32).astype(_f8)

    # xT: [K, IN_F, N], columns in pv[k] order
    xT = np.stack(
        [np.ascontiguousarray(x32[k][pv[k]].T) for k in range(K)]
    ).astype(_bf)


    in_maps = []
    for c in range(NCORES):
        # scatter indices: local pv0-block position of the core's k-row
        # (ns*128+p) -> where to place it in hp_local[k]
        idxS = np.zeros((128, K, RCH), np.int32)
        base = pv[0][c * ROWS:(c + 1) * ROWS]
        blockpos = {n: i for i, n in enumerate(base)}
        for k in range(K):
            lp = np.array([blockpos[n] for n in rowsets[k][c]], np.int32)
            idxS[:, k, :] = lp.reshape(RCH, 128).T
        m = {"xT": xT, "waug": waug, "idxS": idxS}
        xr = np.zeros((K, IN_F, ROWS), np.float32)
        for k in range(K):
            rs = rowsets[k][c]
            xr[k] = x32[k][rs].T
            adjPk = (
                adj8[np.ix_(rs, pv[k])].T     # [N(sorted cols m), ROWS]
                .reshape(NGRP, CPG, 128, ROWS)
                .transpose(0, 2, 1, 3)
            )
            m[f"adjP{k}"] = np.ascontiguousarray(adjPk)
        m["xrT"] = xr.astype(_bf)
        in_maps.append(m)
    return in_maps, cls, rowsets


def _run(in_maps, cls, trace=False, **kw):
    global _build_sched
    from concourse.bass_utils import run_bass_kernel_spmd

    _build_sched = cls
    key = tuple(tuple(tuple(row) for row in kk) for kk in cls)
    nc = _build(key)
    return run_bass_kernel_spmd(nc, in_maps, list(range(NCORES)), trace=trace, **kw)


def kernel(**inputs):
    x = np.asarray(inputs["x"])
    adj = np.asarray(inputs["adj"])
    weight = np.asarray(inputs["weight"])
    a = np.asarray(inputs["a"])
    in_maps, cls, rowsets = _prep(x, adj, weight, a)
    res = _run(in_maps, cls)
    result = np.zeros((K, N, OUT_F), np.float32)
    for c in range(NCORES):
        o = np.asarray(res.results[c]["out"]).reshape(K, OUT_F, ROWS)
        for k in range(K):
            result[k, rowsets[0][c], :] = o[k].T
    return np.ascontiguousarray(result)
